# revision 46
# baseline (speedup 1.0000x reference)
"""Inner-policy-sharded Trainium2 kernel for DecoupledDynamicsModel (MoE).

Model: B=8192 rows; each row selects one of P=8 outer policies via
policy_indices; the selected policy runs 8 inner MLPs (72 -> 512 -> 512 -> 64)
on (latent chunk, action) and the 8 inner outputs concatenate to 512 dims.

Sharding: by INNER policy. Core i computes inner MLP i for every row, using
the row's outer-policy weight set W*[outer, i]. Rows are sorted by outer
policy on the host so tokens form 8 contiguous groups; within a group the
weights are stationary. Perfect load balance (every core runs exactly B
tokens), no capacity padding.

V2 layout (vs the f32r V1, 93230ns -> 86578ns cost-model):
  * All matmul operands are fp16 (1 cycle/row at any moving size, ~5e-4
    rel err): halves every DMA stream and lifts the f32r mult-of-4 /
    >=256-moving restrictions, so tokens need no padding at all.
  * Layer 3 (512 -> 64) runs TRANSPOSED: stationary = h2 token-chunk
    [128h x 128tok], moving = W3 k-chunk [128h x 64]. 16 matmuls of 64
    rows = 1024 cycles per 512-token tile instead of 4 x 512 = 2048 with
    the [64, N] orientation. Output lands token-major [tok, 64] in PSUM.
  * b1 is folded into the L1 matmul as a 73rd contraction row (x row 72
    is all-ones), so L1 evictions are bias-free; tiles <= 256 tokens pack
    multiple L1 m-chunks into one psum bank and evict with a single ACT
    op (kills the serial-eviction stall on small boundary/drain tiles).
  * b3 is not applied on device; the host adds it during the scatter.
  * Output DMAs are batched (4 token-tiles per dma_start): each HWDGE
    dma_start costs ~630ns descriptor-generation + ~900ns completion sem,
    so fewer/larger transfers matter more than latency. All weights and
    x are emitted in the preamble with zero sem waits, so the SP queue
    never head-of-line blocks behind eviction-gated output flushes.
  * Groups are processed in a permuted order chosen so the final group
    has the smallest past-512 remainder -> the drain tile is tiny.
  * A dummy activation at t=0 pulls the ~1.3us ACT function-table load
    into the DMA-startup dead window.

On-chip layout is feature-major ([features(part), tokens(free)]) so no
transposes are needed anywhere. Relu rides the PSUM->SBUF eviction:
ACT evicts layer 1, DVE evicts layers 2 and 3 (bias+relu via
tensor_scalar); both have slack vs the PE, which runs at ~87% occupancy
(75.5us busy, the fp16 4N+16N+2N rows/tile floor). On the last 3 tiles
the m0-2 L2 evictions move to ACT so the L3-gating m3 eviction starts on
DVE unqueued. L3 accumulates all token-chunks in ONE psum bank
(sequential start/stop groups, single eviction after the last matmul, so
no PE-write-vs-read bank hazard). PSUM: 4 L1 + 2 L2 + 2 L3 = 8 banks.
Built on Bacc so multi-wait instructions are legalized.
"""

import sys

sys.path.insert(0, "/opt/trn_rl_repo")

import numpy as np

import concourse.bass as bass
from concourse import bacc
import concourse.mybir as mybir
import concourse.tile as tile
from concourse.bass_utils import run_bass_kernel_spmd

P = 8          # outer policies == n_cores == inner MLPs per policy
Z = 64         # per-policy latent dim
D = P * Z      # 512
A = 8          # action dim
IN = Z + A     # 72, MLP input dim
H = 512        # hidden dim
NCORES = 8

F32 = mybir.dt.float32
F16 = mybir.dt.float16
RELU = mybir.ActivationFunctionType.Relu

TRACE = False
LAST_RESULT = None
YB = 4         # token tiles per batched output DMA

# schedule knobs (sweepable)
CFG = {
    "ps1": 4, "ps2": 2, "ps3": 2,   # PSUM bufs per layer (sum <= 8)
    "l3_pipe": False,                # emit L3(i) after L2(i+1)
    "y_queue": "sync",              # 'sync' | 'gpsimd'
    "ahead": 1,                      # (obsolete: all weights in preamble)
    "x_swdge_all": False,            # all x on SWDGE vs first span only
    "b12_swdge": True,               # biases via SWDGE (frees an HWDGE slot)
    "w2g0_3way": True,               # group-0 w2 as m0|m1|m23 for arrival
    "last_tile": 128,                # final tile size (drain length)
    "hbufs": 4,                      # h1/h2 pool buffers
    "ybufs": 4,                      # y-window pool buffers
    "yb": 4,                         # tiles per y window
    "l2_act": 0,                     # L2 evictions on ACT for m%2<l2_act
    "ytail_act": False,              # final tile y eviction on ACT (Copy)
    "tail_swdge": False,             # final flush DMA on SWDGE
    "esplit_tail": 3,                # m<esplit_m evicts on ACT, last N tiles
    "esplit_m": 3,
    "x0_split": False,               # first SWDGE span = tile0 only
    "b12_late": False,               # b12 after group-0 weights (L1 bias-free)
    "g0_tile": 512,                  # tile cap in the first group
    "end_tile": 512,                 # tile cap in the final group
}


def _group_tiles(counts):
    """Token tiles for the sorted stream: each tile stays inside one outer-
    policy group, <=512 tokens. The very last tile is kept small (<=128) so
    the end-of-kernel serial chain (L2 -> evict -> L3 -> evict -> DMA) is
    short."""
    tiles = []
    ng = len(counts)
    for g, n in enumerate(counts):
        off = sum(counts[:g])
        r = n
        first = True
        while r > 0:
            cap = 512
            if g == 0:
                cap = CFG["g0_tile"]
            elif g == ng - 1:
                cap = CFG["end_tile"]
            t = min(r, cap)
            # small leading tile: the first matmul is gated on a tiny x DMA
            if g == 0 and first and r > 384:
                t = 128
            # shorten the drain: final chunk of the final group is small
            lt = CFG["last_tile"]
            if g == ng - 1 and r <= cap and r > lt:
                t = min(t, r - lt)
            tiles.append((g, off, t))
            off += t
            r -= t
            first = False
    return tiles


def _build_program(counts, B):
    tiles = _group_tiles(counts)
    ntiles = len(tiles)
    YB = CFG["yb"]
    nc = bacc.Bacc()

    xTd = nc.declare_dram_parameter("xT", [IN + 1, B], F16, isOutput=False)
    w1d = nc.declare_dram_parameter("w1", [IN + 1, P, 512], F16, isOutput=False)
    w2d = nc.declare_dram_parameter("w2", [128, P, 4, 512], F16, isOutput=False)
    w3d = nc.declare_dram_parameter("w3", [128, P, 4, Z], F16, isOutput=False)
    b12d = nc.declare_dram_parameter("b12", [128, P, 8], F32, isOutput=False)
    # partition-major scrambled output: [partition, tile, tok-chunk, z]
    yd = nc.declare_dram_parameter("yS", [128, ntiles, 4, Z], F16, isOutput=True)

    # group boundaries in the sorted token stream
    gstart = [sum(counts[:g]) for g in range(P)]
    gend = [gstart[g] + counts[g] for g in range(P)]

    with tile.TileContext(nc) as tc:
        with (
            tc.tile_pool(name="w1p", bufs=8) as w1p,
            tc.tile_pool(name="w2p", bufs=8) as w2p,
            tc.tile_pool(name="w3p", bufs=8) as w3p,
            tc.tile_pool(name="bp", bufs=1) as bp,
            tc.tile_pool(name="xs", bufs=1) as xpool,
            tc.tile_pool(name="hs", bufs=CFG["hbufs"]) as hpool,
            tc.tile_pool(name="ys", bufs=CFG["ybufs"]) as ypool,
            tc.tile_pool(name="ps1", bufs=CFG["ps1"], space="PSUM") as pspool1,
            tc.tile_pool(name="ps2", bufs=CFG["ps2"], space="PSUM") as pspool2,
            tc.tile_pool(name="ps3", bufs=CFG["ps3"], space="PSUM") as pspool3,
        ):
            xt = xpool.tile([IN + 1, B], F16, tag="x")
            b12 = bp.tile([128, P, 8], F32, tag="b12")
            # dummy activation right at t=0: pulls the ~1.3us ACT
            # function-table load into the DMA-latency dead window
            warm = bp.tile([1, 1], F32, tag="warm")
            nc.vector.memset(warm, 0.0)
            nc.scalar.activation(warm[:, :], warm[:, :], RELU)
            w1s = [w1p.tile([IN + 1, 512], F16, tag="w1", name=f"w1_{g}") for g in range(P)]
            w2s = [w2p.tile([128, 4, 512], F16, tag="w2", name=f"w2_{g}") for g in range(P)]
            w3s = [w3p.tile([128, 4, Z], F16, tag="w3", name=f"w3_{g}") for g in range(P)]

            # --- DMA emission in need order ------------------------------
            # Every HWDGE dma_start costs ~630ns on the (serial) descriptor
            # generator plus ~900ns completion-sem latency, so batch
            # transfers wherever latency allows. ALL x columns ride SWDGE
            # (gpsimd), in spans sized so each tile's tokens land just
            # before its L1; the HWDGE pipe carries only weights + outputs.
            if CFG["x_swdge_all"]:
                xcuts = [0, tiles[0][2], 640, 2048, 4608, B]
                xcuts = sorted(set(min(c, B) for c in xcuts))
                for c0, c1 in zip(xcuts[:-1], xcuts[1:]):
                    if c1 > c0:
                        nc.gpsimd.dma_start(xt[:, c0:c1], xTd[:, c0:c1])
            else:
                sw_end = min(640, gend[0])
                if CFG["x0_split"] and tiles[0][2] < sw_end:
                    c = tiles[0][2]
                    nc.gpsimd.dma_start(xt[:, 0:c], xTd[:, 0:c])
                    nc.gpsimd.dma_start(xt[:, c:sw_end], xTd[:, c:sw_end])
                else:
                    nc.gpsimd.dma_start(xt[:, 0:sw_end], xTd[:, 0:sw_end])

            def emit_group_weights(g, split_w2=False):
                nc.sync.dma_start(w1s[g][:, :], w1d[:, g, :])
                if split_w2:
                    nc.sync.dma_start(w2s[g][:, 0, :], w2d[:, g, 0, :])
                    nc.sync.dma_start(w2s[g][:, 1:4, :], w2d[:, g, 1:4, :])
                else:
                    nc.sync.dma_start(w2s[g][:, :, :], w2d[:, g, :, :])
                nc.sync.dma_start(w3s[g][:, :, :], w3d[:, g, :, :])

            def emit_group_x(g):
                if not CFG["x_swdge_all"] and g < P:
                    lo = max(gstart[g], min(640, gend[0]) if g == 0 else gstart[g])
                    if g == 0:
                        lo = min(640, gend[0])
                    if lo < gend[g]:
                        nc.sync.dma_start(xt[:, lo:gend[g]], xTd[:, lo:gend[g]])

            # ENTIRE weight + x stream in the preamble: none of these DMAs
            # has a sem wait, so the SP queue never head-of-line blocks and
            # every group's weights are resident long before needed.
            nc.sync.dma_start(w1s[0][:, :], w1d[:, 0, :])
            if not CFG["b12_late"]:
                if CFG["b12_swdge"]:
                    nc.gpsimd.dma_start(b12[:, :, :], b12d[:, :, :])
                else:
                    nc.sync.dma_start(b12[:, :, :], b12d[:, :, :])
            nc.sync.dma_start(w2s[0][:, 0, :], w2d[:, 0, 0, :])
            if CFG["w2g0_3way"]:
                nc.sync.dma_start(w2s[0][:, 1, :], w2d[:, 0, 1, :])
                nc.sync.dma_start(w2s[0][:, 2:4, :], w2d[:, 0, 2:4, :])
            else:
                nc.sync.dma_start(w2s[0][:, 1:4, :], w2d[:, 0, 1:4, :])
            nc.sync.dma_start(w3s[0][:, :, :], w3d[:, 0, :, :])
            if CFG["b12_late"]:
                nc.sync.dma_start(b12[:, :, :], b12d[:, :, :])
            emit_group_x(0)
            for g in range(1, P):
                emit_group_x(g)
                emit_group_weights(g)

            # --- compute loop --------------------------------------------
            # Layer 3 of tile i is emitted after layer 2 of tile i+1: its
            # stationary operand is h2 (produced by the DVE eviction), so
            # the one-tile delay hides the eviction latency from the PE.
            # y-window boundaries: every YB tiles, plus a forced boundary so
            # the final (tiny) tile flushes alone -> short end-of-program DMA
            flush_after = {ti for ti in range(ntiles) if ti % YB == YB - 1}
            flush_after.add(ntiles - 1)
            if ntiles >= 2:
                flush_after.add(ntiles - 2)
            ystate = {"yt": None, "flushed": -1}

            def emit_l3(pg, ptw, ph2, pti):
                if pti % YB == 0:
                    ystate["yt"] = ypool.tile(
                        [128, YB, 4, Z], F16, tag="y", name=f"yt_{pti // YB}"
                    )
                yt = ystate["yt"]
                nch = -(-ptw // 128)
                # all tok-chunks accumulate into ONE psum bank (sequential
                # start/stop groups, no eviction until every chunk is done,
                # so no PE-write-vs-engine-read bank hazard), then a single
                # DVE eviction moves the whole [128, nch, 64] block out
                ps = pspool3.tile([128, 4, Z], F32, tag="ps3",
                                  name=f"ps3_{pti}")
                for c in range(nch):
                    cw = min(128, ptw - c * 128)
                    for k in range(4):
                        nc.tensor.matmul(
                            ps[0:cw, c, :],
                            ph2[:, k, c * 128:c * 128 + cw],
                            w3s[pg][:, k, :],
                            start=(k == 0),
                            stop=(k == 3),
                        )
                nc.vector.tensor_scalar(
                    yt[:, pti % YB, 0:nch, :], ps[:, 0:nch, :], 0.0, None,
                    mybir.AluOpType.add,
                )
                if pti in flush_after:
                    t_lo = ystate["flushed"] + 1
                    nb = pti - t_lo + 1
                    if nb > 0:
                        # mid-run flushes ride SWDGE (nothing queued behind
                        # their eviction waits); the final flush uses the
                        # now-empty SP queue for its lower latency
                        last_flush = pti == ntiles - 1
                        if last_flush:
                            dma = (nc.gpsimd.dma_start if CFG["tail_swdge"]
                                   else nc.sync.dma_start)
                        else:
                            dma = (nc.gpsimd.dma_start
                                   if CFG["y_queue"] == "gpsimd"
                                   else nc.sync.dma_start)
                        dma(
                            yd[:, t_lo:pti + 1, :, :],
                            yt[:, t_lo % YB:t_lo % YB + nb, :, :],
                        )
                        ystate["flushed"] = pti

            pending = None
            prev_g = 0
            for (ti, (g, t0, tw)) in enumerate(tiles):


                h1 = hpool.tile([128, 4, 512], F16, tag="h1")
                h2 = hpool.tile([128, 4, 512], F16, tag="h2")
                # layer 1: h1 = relu(W1'.T @ x'), b1 folded in as a 73rd
                # contraction row (x row 72 is all-ones)
                if tw <= 256:
                    # small tile: m-chunks share psum banks, merged evicts
                    mper = 4 if tw <= 128 else 2
                    for m0 in range(0, 4, mper):
                        ps = pspool1.tile([128, mper, tw], F32, tag="ps1",
                                          name=f"ps1m_{ti}_{m0}")
                        for j in range(mper):
                            nc.tensor.matmul(
                                ps[:, j, :],
                                w1s[g][:, (m0 + j) * 128:(m0 + j + 1) * 128],
                                xt[:, t0:t0 + tw],
                                start=True,
                                stop=True,
                            )
                        nc.scalar.activation(
                            h1[:, m0:m0 + mper, :tw], ps[:, :, :], RELU
                        )
                else:
                    for m in range(4):
                        ps = pspool1.tile([128, 512], F32, tag="ps1")
                        nc.tensor.matmul(
                            ps[:, :tw],
                            w1s[g][:, m * 128:(m + 1) * 128],
                            xt[:, t0:t0 + tw],
                            start=True,
                            stop=True,
                        )
                        nc.scalar.activation(h1[:, m, :tw], ps[:, :tw], RELU)
                # layer 2: h2 = relu(W2.T @ h1 + b2), K=512 over 4 chunks
                for m in range(4):
                    ps = pspool2.tile([128, 512], F32, tag="ps2")
                    for k in range(4):
                        nc.tensor.matmul(
                            ps[:, :tw],
                            w2s[g][:, m, k * 128:(k + 1) * 128],
                            h1[:, k, :tw],
                            start=(k == 0),
                            stop=(k == 3),
                        )
                    if ti >= ntiles - CFG["esplit_tail"] and m < CFG["esplit_m"]:
                        # endgame: m0-2 evictions ride the (idle) ACT so the
                        # L3-gating m3 eviction starts on DVE with no queue
                        nc.scalar.activation(
                            h2[:, m, :tw], ps[:, :tw], RELU,
                            bias=b12[:, g, 4 + m:5 + m],
                        )
                    elif m % 2 < CFG["l2_act"]:
                        nc.scalar.activation(
                            h2[:, m, :tw], ps[:, :tw], RELU,
                            bias=b12[:, g, 4 + m:5 + m],
                        )
                    else:
                        nc.vector.tensor_scalar(
                            h2[:, m, :tw],
                            ps[:, :tw],
                            b12[:, g, 4 + m:5 + m],
                            0.0,
                            mybir.AluOpType.add,
                            mybir.AluOpType.max,
                        )
                # layer 3 of the previous (or current) tile, see emit_l3
                if CFG["l3_pipe"]:
                    if pending is not None:
                        emit_l3(*pending)
                    pending = (g, tw, h2, ti)
                else:
                    emit_l3(g, tw, h2, ti)
            if pending is not None:
                emit_l3(*pending)

    nc.finalize()
    return nc, tiles


def _pack_inputs(latents, actions, order, counts, order_g, W1, b1, W2, b2, W3):
    """Per-core inputs. Core i: xT = [latent chunk i; action] for all rows in
    sorted order; weight tensors hold (outer g, inner i) stacks."""
    B = latents.shape[0]
    gsel = np.asarray(order_g)
    W1, W2, W3, b1, b2 = W1[gsel], W2[gsel], W3[gsel], b1[gsel], b2[gsel]
    lat_s = latents[order]                       # [B, 512]
    act_s = actions[order]                       # [B, 8]
    in_maps = []
    for i in range(NCORES):
        xT = np.empty((IN + 1, B), dtype=np.float16)
        xT[:Z] = lat_s[:, i * Z:(i + 1) * Z].T
        xT[Z:IN] = act_s.T
        xT[IN] = 1.0

        w1 = np.empty((IN + 1, P, 512), dtype=np.float16)
        w1[:IN] = W1[:, i].transpose(1, 0, 2).astype(np.float16)
        w1[IN] = b1[:, i].astype(np.float16)     # b1 folded as K-row 72
        # w2[p, g, m, k*128+q] = W2[g, i, k*128+p, m*128+q]
        w2 = np.ascontiguousarray(
            W2[:, i].reshape(P, 4, 128, 4, 128)   # g, k, p, m, q
            .transpose(2, 0, 3, 1, 4)             # p, g, m, k, q
            .reshape(128, P, 4, 512)
        ).astype(np.float16)
        # w3[p, g, k, z] = W3[g, i, k*128+p, z]
        w3 = np.ascontiguousarray(
            W3[:, i].reshape(P, 4, 128, Z).transpose(2, 0, 1, 3)
        ).astype(np.float16)
        b12 = np.empty((128, P, 8), dtype=np.float32)
        b12[:, :, 0:4] = b1[:, i].reshape(P, 4, 128).transpose(2, 0, 1)
        b12[:, :, 4:8] = b2[:, i].reshape(P, 4, 128).transpose(2, 0, 1)

        in_maps.append({"xT": xT, "w1": w1, "w2": w2, "w3": w3, "b12": b12})
    return in_maps


def _prepare(latents, actions, policy_indices, W1, b1, W2, b2, W3, b3):
    latents = np.asarray(latents, dtype=np.float32)
    actions = np.asarray(actions, dtype=np.float32)
    idx = np.asarray(policy_indices).astype(np.int64)
    W1 = np.ascontiguousarray(np.asarray(W1, dtype=np.float32))
    W2 = np.ascontiguousarray(np.asarray(W2, dtype=np.float32))
    W3 = np.ascontiguousarray(np.asarray(W3, dtype=np.float32))
    b1 = np.asarray(b1, dtype=np.float32)
    b2 = np.asarray(b2, dtype=np.float32)

    counts0 = np.bincount(idx, minlength=P)
    # group processing order: the group with the smallest past-512 remainder
    # runs LAST, so the endgame is [512, 512, tiny] with a minimal drain
    rem = [int(c) % 512 if int(c) % 512 else 512 for c in counts0]
    last = int(np.argmin(rem))
    order_g = [g for g in range(P) if g != last] + [last]
    rank = np.empty(P, dtype=np.int64)
    rank[order_g] = np.arange(P)
    order = np.argsort(rank[idx], kind="stable")
    counts = [int(counts0[g]) for g in order_g]

    in_maps = _pack_inputs(
        latents, actions, order, counts, order_g, W1, b1, W2, b2, W3
    )
    nc, tiles = _build_program(counts, latents.shape[0])
    return nc, in_maps, order, counts, tiles


def _scatter_out(results, order, tiles, B, policy_indices, b3):
    out = np.empty((B, D), dtype=np.float32)
    for i in range(NCORES):
        yS = np.asarray(results[i]["yS"], dtype=np.float32)  # [128, NT, 4, 64]
        ys = np.empty((B, Z), dtype=np.float32)
        for ti, (g, t0, tw) in enumerate(tiles):
            nch = -(-tw // 128)
            for c in range(nch):
                cw = min(128, tw - c * 128)
                ys[t0 + c * 128:t0 + c * 128 + cw] = yS[0:cw, ti, c, :]
        out[order, i * Z:(i + 1) * Z] = ys
    idx = np.asarray(policy_indices).astype(np.int64)
    out += np.asarray(b3, dtype=np.float32)[idx].reshape(B, D)
    return out


def kernel(latents, actions, policy_indices, W1, b1, W2, b2, W3, b3):
    global LAST_RESULT
    nc, in_maps, order, counts, tiles = _prepare(
        latents, actions, policy_indices, W1, b1, W2, b2, W3, b3
    )
    res = run_bass_kernel_spmd(nc, in_maps, list(range(NCORES)), trace=TRACE)
    LAST_RESULT = res
    return _scatter_out(
        res.results, order, tiles, np.asarray(latents).shape[0],
        policy_indices, b3,
    )


# revision 50
# speedup vs baseline: 1.0008x; 1.0008x over previous
"""Inner-policy-sharded Trainium2 kernel for DecoupledDynamicsModel (MoE).

Model: B=8192 rows; each row selects one of P=8 outer policies via
policy_indices; the selected policy runs 8 inner MLPs (72 -> 512 -> 512 -> 64)
on (latent chunk, action) and the 8 inner outputs concatenate to 512 dims.

Sharding: by INNER policy. Core i computes inner MLP i for every row, using
the row's outer-policy weight set W*[outer, i]. Rows are sorted by outer
policy on the host so tokens form 8 contiguous groups; within a group the
weights are stationary. Perfect load balance (every core runs exactly B
tokens), no capacity padding.

V2 layout (vs the f32r V1, 93230ns -> 86578ns cost-model):
  * All matmul operands are fp16 (1 cycle/row at any moving size, ~5e-4
    rel err): halves every DMA stream and lifts the f32r mult-of-4 /
    >=256-moving restrictions, so tokens need no padding at all.
  * Layer 3 (512 -> 64) runs TRANSPOSED: stationary = h2 token-chunk
    [128h x 128tok], moving = W3 k-chunk [128h x 64]. 16 matmuls of 64
    rows = 1024 cycles per 512-token tile instead of 4 x 512 = 2048 with
    the [64, N] orientation. Output lands token-major [tok, 64] in PSUM.
  * b1 is folded into the L1 matmul as a 73rd contraction row (x row 72
    is all-ones), so L1 evictions are bias-free; tiles <= 256 tokens pack
    multiple L1 m-chunks into one psum bank and evict with a single ACT
    op (kills the serial-eviction stall on small boundary/drain tiles).
  * b3 is not applied on device; the host adds it during the scatter.
  * Output DMAs are batched (4 token-tiles per dma_start): each HWDGE
    dma_start costs ~630ns descriptor-generation + ~900ns completion sem,
    so fewer/larger transfers matter more than latency. All weights and
    x are emitted in the preamble with zero sem waits, so the SP queue
    never head-of-line blocks behind eviction-gated output flushes.
  * Groups are processed in a permuted order chosen so the final group
    has the smallest past-512 remainder -> the drain tile is tiny.
  * A dummy activation at t=0 pulls the ~1.3us ACT function-table load
    into the DMA-startup dead window.

On-chip layout is feature-major ([features(part), tokens(free)]) so no
transposes are needed anywhere. Relu rides the PSUM->SBUF eviction:
ACT evicts layer 1, DVE evicts layers 2 and 3 (bias+relu via
tensor_scalar); both have slack vs the PE, which runs at ~87% occupancy
(75.5us busy, the fp16 4N+16N+2N rows/tile floor). On the last 3 tiles
the m0-2 L2 evictions move to ACT so the L3-gating m3 eviction starts on
DVE unqueued. L3 accumulates all token-chunks in ONE psum bank
(sequential start/stop groups, single eviction after the last matmul, so
no PE-write-vs-read bank hazard). PSUM: 4 L1 + 2 L2 + 2 L3 = 8 banks.
Built on Bacc so multi-wait instructions are legalized.
"""

import sys

sys.path.insert(0, "/opt/trn_rl_repo")

import numpy as np

import concourse.bass as bass
from concourse import bacc
import concourse.mybir as mybir
import concourse.tile as tile
from concourse.bass_utils import run_bass_kernel_spmd

P = 8          # outer policies == n_cores == inner MLPs per policy
Z = 64         # per-policy latent dim
D = P * Z      # 512
A = 8          # action dim
IN = Z + A     # 72, MLP input dim
H = 512        # hidden dim
NCORES = 8

F32 = mybir.dt.float32
F16 = mybir.dt.float16
RELU = mybir.ActivationFunctionType.Relu

TRACE = False
LAST_RESULT = None
YB = 4         # token tiles per batched output DMA

# schedule knobs (sweepable)
CFG = {
    "ps1": 4, "ps2": 2, "ps3": 2,   # PSUM bufs per layer (sum <= 8)
    "l3_pipe": False,                # emit L3(i) after L2(i+1)
    "y_queue": "sync",              # 'sync' | 'gpsimd'
    "ahead": 1,                      # (obsolete: all weights in preamble)
    "x_swdge_all": False,            # all x on SWDGE vs first span only
    "b12_swdge": True,               # biases via SWDGE (frees an HWDGE slot)
    "w2g0_3way": True,               # group-0 w2 as m0|m1|m23 for arrival
    "last_tile": 128,                # final tile size (drain length)
    "hbufs": 4,                      # h1/h2 pool buffers
    "ybufs": 4,                      # y-window pool buffers
    "yb": 4,                         # tiles per y window
    "l2_act": 0,                     # L2 evictions on ACT for m%2<l2_act
    "ytail_act": False,              # final tile y eviction on ACT (Copy)
    "tail_swdge": False,             # final flush DMA on SWDGE
    "esplit_tail": 3,                # m<esplit_m evicts on ACT, last N tiles
    "esplit_m": 3,
    "x0_split": False,               # first SWDGE span = tile0 only
    "l3_pipe_n": 3,                  # L3-pipe only the last N tiles
    "b12_late": False,               # b12 after group-0 weights (L1 bias-free)
    "g0_tile": 512,                  # tile cap in the first group
    "end_tile": 512,                 # tile cap in the final group
}


def _group_tiles(counts):
    """Token tiles for the sorted stream: each tile stays inside one outer-
    policy group, <=512 tokens. The very last tile is kept small (<=128) so
    the end-of-kernel serial chain (L2 -> evict -> L3 -> evict -> DMA) is
    short."""
    tiles = []
    ng = len(counts)
    for g, n in enumerate(counts):
        off = sum(counts[:g])
        r = n
        first = True
        while r > 0:
            cap = 512
            if g == 0:
                cap = CFG["g0_tile"]
            elif g == ng - 1:
                cap = CFG["end_tile"]
            t = min(r, cap)
            # small leading tile: the first matmul is gated on a tiny x DMA
            if g == 0 and first and r > 384:
                t = 128
            # shorten the drain: final chunk of the final group is small
            lt = CFG["last_tile"]
            if g == ng - 1 and r <= cap and r > lt:
                t = min(t, r - lt)
            tiles.append((g, off, t))
            off += t
            r -= t
            first = False
    return tiles


def _build_program(counts, B):
    tiles = _group_tiles(counts)
    ntiles = len(tiles)
    YB = CFG["yb"]
    nc = bacc.Bacc()

    xTd = nc.declare_dram_parameter("xT", [IN + 1, B], F16, isOutput=False)
    w1d = nc.declare_dram_parameter("w1", [IN + 1, P, 512], F16, isOutput=False)
    w2d = nc.declare_dram_parameter("w2", [128, P, 4, 512], F16, isOutput=False)
    w3d = nc.declare_dram_parameter("w3", [128, P, 4, Z], F16, isOutput=False)
    b12d = nc.declare_dram_parameter("b12", [128, P, 8], F32, isOutput=False)
    # partition-major scrambled output: [partition, tile, tok-chunk, z]
    yd = nc.declare_dram_parameter("yS", [128, ntiles, 4, Z], F16, isOutput=True)

    # group boundaries in the sorted token stream
    gstart = [sum(counts[:g]) for g in range(P)]
    gend = [gstart[g] + counts[g] for g in range(P)]

    with tile.TileContext(nc) as tc:
        with (
            tc.tile_pool(name="w1p", bufs=8) as w1p,
            tc.tile_pool(name="w2p", bufs=8) as w2p,
            tc.tile_pool(name="w3p", bufs=8) as w3p,
            tc.tile_pool(name="bp", bufs=1) as bp,
            tc.tile_pool(name="xs", bufs=1) as xpool,
            tc.tile_pool(name="hs", bufs=CFG["hbufs"]) as hpool,
            tc.tile_pool(name="ys", bufs=CFG["ybufs"]) as ypool,
            tc.tile_pool(name="ps1", bufs=CFG["ps1"], space="PSUM") as pspool1,
            tc.tile_pool(name="ps2", bufs=CFG["ps2"], space="PSUM") as pspool2,
            tc.tile_pool(name="ps3", bufs=CFG["ps3"], space="PSUM") as pspool3,
        ):
            xt = xpool.tile([IN + 1, B], F16, tag="x")
            b12 = bp.tile([128, P, 8], F32, tag="b12")
            # dummy activation right at t=0: pulls the ~1.3us ACT
            # function-table load into the DMA-latency dead window
            warm = bp.tile([1, 1], F32, tag="warm")
            nc.vector.memset(warm, 0.0)
            nc.scalar.activation(warm[:, :], warm[:, :], RELU)
            w1s = [w1p.tile([IN + 1, 512], F16, tag="w1", name=f"w1_{g}") for g in range(P)]
            w2s = [w2p.tile([128, 4, 512], F16, tag="w2", name=f"w2_{g}") for g in range(P)]
            w3s = [w3p.tile([128, 4, Z], F16, tag="w3", name=f"w3_{g}") for g in range(P)]

            # --- DMA emission in need order ------------------------------
            # Every HWDGE dma_start costs ~630ns on the (serial) descriptor
            # generator plus ~900ns completion-sem latency, so batch
            # transfers wherever latency allows. ALL x columns ride SWDGE
            # (gpsimd), in spans sized so each tile's tokens land just
            # before its L1; the HWDGE pipe carries only weights + outputs.
            if CFG["x_swdge_all"]:
                xcuts = [0, tiles[0][2], 640, 2048, 4608, B]
                xcuts = sorted(set(min(c, B) for c in xcuts))
                for c0, c1 in zip(xcuts[:-1], xcuts[1:]):
                    if c1 > c0:
                        nc.gpsimd.dma_start(xt[:, c0:c1], xTd[:, c0:c1])
            else:
                sw_end = min(640, gend[0])
                if CFG["x0_split"] and tiles[0][2] < sw_end:
                    c = tiles[0][2]
                    nc.gpsimd.dma_start(xt[:, 0:c], xTd[:, 0:c])
                    nc.gpsimd.dma_start(xt[:, c:sw_end], xTd[:, c:sw_end])
                else:
                    nc.gpsimd.dma_start(xt[:, 0:sw_end], xTd[:, 0:sw_end])

            def emit_group_weights(g, split_w2=False):
                nc.sync.dma_start(w1s[g][:, :], w1d[:, g, :])
                if split_w2:
                    nc.sync.dma_start(w2s[g][:, 0, :], w2d[:, g, 0, :])
                    nc.sync.dma_start(w2s[g][:, 1:4, :], w2d[:, g, 1:4, :])
                else:
                    nc.sync.dma_start(w2s[g][:, :, :], w2d[:, g, :, :])
                nc.sync.dma_start(w3s[g][:, :, :], w3d[:, g, :, :])

            def emit_group_x(g):
                if not CFG["x_swdge_all"] and g < P:
                    lo = max(gstart[g], min(640, gend[0]) if g == 0 else gstart[g])
                    if g == 0:
                        lo = min(640, gend[0])
                    if lo < gend[g]:
                        nc.sync.dma_start(xt[:, lo:gend[g]], xTd[:, lo:gend[g]])

            # ENTIRE weight + x stream in the preamble: none of these DMAs
            # has a sem wait, so the SP queue never head-of-line blocks and
            # every group's weights are resident long before needed.
            nc.sync.dma_start(w1s[0][:, :], w1d[:, 0, :])
            if not CFG["b12_late"]:
                if CFG["b12_swdge"]:
                    nc.gpsimd.dma_start(b12[:, :, :], b12d[:, :, :])
                else:
                    nc.sync.dma_start(b12[:, :, :], b12d[:, :, :])
            nc.sync.dma_start(w2s[0][:, 0, :], w2d[:, 0, 0, :])
            if CFG["w2g0_3way"]:
                nc.sync.dma_start(w2s[0][:, 1, :], w2d[:, 0, 1, :])
                nc.sync.dma_start(w2s[0][:, 2:4, :], w2d[:, 0, 2:4, :])
            else:
                nc.sync.dma_start(w2s[0][:, 1:4, :], w2d[:, 0, 1:4, :])
            nc.sync.dma_start(w3s[0][:, :, :], w3d[:, 0, :, :])
            if CFG["b12_late"]:
                nc.sync.dma_start(b12[:, :, :], b12d[:, :, :])
            emit_group_x(0)
            for g in range(1, P):
                emit_group_x(g)
                emit_group_weights(g)

            # --- compute loop --------------------------------------------
            # Layer 3 of tile i is emitted after layer 2 of tile i+1: its
            # stationary operand is h2 (produced by the DVE eviction), so
            # the one-tile delay hides the eviction latency from the PE.
            # y-window boundaries: every YB tiles, plus a forced boundary so
            # the final (tiny) tile flushes alone -> short end-of-program DMA
            flush_after = {ti for ti in range(ntiles) if ti % YB == YB - 1}
            flush_after.add(ntiles - 1)
            if ntiles >= 2:
                flush_after.add(ntiles - 2)
            ystate = {"yt": None, "flushed": -1}

            def emit_l3(pg, ptw, ph2, pti):
                if pti % YB == 0:
                    ystate["yt"] = ypool.tile(
                        [128, YB, 4, Z], F16, tag="y", name=f"yt_{pti // YB}"
                    )
                yt = ystate["yt"]
                nch = -(-ptw // 128)
                # all tok-chunks accumulate into ONE psum bank (sequential
                # start/stop groups, no eviction until every chunk is done,
                # so no PE-write-vs-engine-read bank hazard), then a single
                # DVE eviction moves the whole [128, nch, 64] block out
                ps = pspool3.tile([128, 4, Z], F32, tag="ps3",
                                  name=f"ps3_{pti}")
                for c in range(nch):
                    cw = min(128, ptw - c * 128)
                    for k in range(4):
                        nc.tensor.matmul(
                            ps[0:cw, c, :],
                            ph2[:, k, c * 128:c * 128 + cw],
                            w3s[pg][:, k, :],
                            start=(k == 0),
                            stop=(k == 3),
                        )
                nc.vector.tensor_scalar(
                    yt[:, pti % YB, 0:nch, :], ps[:, 0:nch, :], 0.0, None,
                    mybir.AluOpType.add,
                )
                if pti in flush_after:
                    t_lo = ystate["flushed"] + 1
                    nb = pti - t_lo + 1
                    if nb > 0:
                        # mid-run flushes ride SWDGE (nothing queued behind
                        # their eviction waits); the final flush uses the
                        # now-empty SP queue for its lower latency
                        last_flush = pti == ntiles - 1
                        if last_flush:
                            dma = (nc.gpsimd.dma_start if CFG["tail_swdge"]
                                   else nc.sync.dma_start)
                        else:
                            dma = (nc.gpsimd.dma_start
                                   if CFG["y_queue"] == "gpsimd"
                                   else nc.sync.dma_start)
                        dma(
                            yd[:, t_lo:pti + 1, :, :],
                            yt[:, t_lo % YB:t_lo % YB + nb, :, :],
                        )
                        ystate["flushed"] = pti

            def emit_l1(g, t0, tw, ti):
                """Layer 1: h1 = relu(W1'.T @ x'), b1 folded in as the 73rd
                contraction row (x row 72 is all-ones). Returns the h1 tile."""
                h1 = hpool.tile([128, 4, 512], F16, tag="h1", name=f"h1_{ti}")
                if tw <= 256:
                    # small tile: m-chunks share psum banks, merged evicts
                    mper = 4 if tw <= 128 else 2
                    for m0 in range(0, 4, mper):
                        ps = pspool1.tile([128, mper, tw], F32, tag="ps1",
                                          name=f"ps1m_{ti}_{m0}")
                        for j in range(mper):
                            nc.tensor.matmul(
                                ps[:, j, :],
                                w1s[g][:, (m0 + j) * 128:(m0 + j + 1) * 128],
                                xt[:, t0:t0 + tw],
                                start=True,
                                stop=True,
                            )
                        nc.scalar.activation(
                            h1[:, m0:m0 + mper, :tw], ps[:, :, :], RELU
                        )
                else:
                    for m in range(4):
                        ps = pspool1.tile([128, 512], F32, tag="ps1",
                                          name=f"ps1_{ti}_{m}")
                        nc.tensor.matmul(
                            ps[:, :tw],
                            w1s[g][:, m * 128:(m + 1) * 128],
                            xt[:, t0:t0 + tw],
                            start=True,
                            stop=True,
                        )
                        nc.scalar.activation(h1[:, m, :tw], ps[:, :tw], RELU)
                return h1

            hoisted_h1 = {}
            pending = None
            prev_g = 0
            for (ti, (g, t0, tw)) in enumerate(tiles):


                h2 = hpool.tile([128, 4, 512], F16, tag="h2")
                if ti in hoisted_h1:
                    h1 = hoisted_h1.pop(ti)
                else:
                    h1 = emit_l1(g, t0, tw, ti)
                # hoist the drain tile's (tiny) L1 one tile early: it only
                # needs x + w1, and its ACT eviction then hides under this
                # tile's L2 compute instead of gating the drain tile's L2
                if ti + 1 == ntiles - 1 and ntiles >= 2:
                    ng, nt0, ntw = tiles[ti + 1]
                    hoisted_h1[ti + 1] = emit_l1(ng, nt0, ntw, ti + 1)
                # layer 2: h2 = relu(W2.T @ h1 + b2), K=512 over 4 chunks
                for m in range(4):
                    ps = pspool2.tile([128, 512], F32, tag="ps2")
                    for k in range(4):
                        nc.tensor.matmul(
                            ps[:, :tw],
                            w2s[g][:, m, k * 128:(k + 1) * 128],
                            h1[:, k, :tw],
                            start=(k == 0),
                            stop=(k == 3),
                        )
                    if ti >= ntiles - CFG["esplit_tail"] and m < CFG["esplit_m"]:
                        # endgame: m0-2 evictions ride the (idle) ACT so the
                        # L3-gating m3 eviction starts on DVE with no queue
                        nc.scalar.activation(
                            h2[:, m, :tw], ps[:, :tw], RELU,
                            bias=b12[:, g, 4 + m:5 + m],
                        )
                    elif m % 2 < CFG["l2_act"]:
                        nc.scalar.activation(
                            h2[:, m, :tw], ps[:, :tw], RELU,
                            bias=b12[:, g, 4 + m:5 + m],
                        )
                    else:
                        nc.vector.tensor_scalar(
                            h2[:, m, :tw],
                            ps[:, :tw],
                            b12[:, g, 4 + m:5 + m],
                            0.0,
                            mybir.AluOpType.add,
                            mybir.AluOpType.max,
                        )
                # layer 3 of the previous (or current) tile, see emit_l3
                if CFG["l3_pipe"] or ti >= ntiles - CFG["l3_pipe_n"]:
                    if pending is not None:
                        emit_l3(*pending)
                    pending = (g, tw, h2, ti)
                else:
                    emit_l3(g, tw, h2, ti)
            if pending is not None:
                emit_l3(*pending)

    nc.finalize()
    return nc, tiles


def _pack_inputs(latents, actions, order, counts, order_g, W1, b1, W2, b2, W3):
    """Per-core inputs. Core i: xT = [latent chunk i; action] for all rows in
    sorted order; weight tensors hold (outer g, inner i) stacks."""
    B = latents.shape[0]
    gsel = np.asarray(order_g)
    W1, W2, W3, b1, b2 = W1[gsel], W2[gsel], W3[gsel], b1[gsel], b2[gsel]
    lat_s = latents[order]                       # [B, 512]
    act_s = actions[order]                       # [B, 8]
    in_maps = []
    for i in range(NCORES):
        xT = np.empty((IN + 1, B), dtype=np.float16)
        xT[:Z] = lat_s[:, i * Z:(i + 1) * Z].T
        xT[Z:IN] = act_s.T
        xT[IN] = 1.0

        w1 = np.empty((IN + 1, P, 512), dtype=np.float16)
        w1[:IN] = W1[:, i].transpose(1, 0, 2).astype(np.float16)
        w1[IN] = b1[:, i].astype(np.float16)     # b1 folded as K-row 72
        # w2[p, g, m, k*128+q] = W2[g, i, k*128+p, m*128+q]
        w2 = np.ascontiguousarray(
            W2[:, i].reshape(P, 4, 128, 4, 128)   # g, k, p, m, q
            .transpose(2, 0, 3, 1, 4)             # p, g, m, k, q
            .reshape(128, P, 4, 512)
        ).astype(np.float16)
        # w3[p, g, k, z] = W3[g, i, k*128+p, z]
        w3 = np.ascontiguousarray(
            W3[:, i].reshape(P, 4, 128, Z).transpose(2, 0, 1, 3)
        ).astype(np.float16)
        b12 = np.empty((128, P, 8), dtype=np.float32)
        b12[:, :, 0:4] = b1[:, i].reshape(P, 4, 128).transpose(2, 0, 1)
        b12[:, :, 4:8] = b2[:, i].reshape(P, 4, 128).transpose(2, 0, 1)

        in_maps.append({"xT": xT, "w1": w1, "w2": w2, "w3": w3, "b12": b12})
    return in_maps


def _prepare(latents, actions, policy_indices, W1, b1, W2, b2, W3, b3):
    latents = np.asarray(latents, dtype=np.float32)
    actions = np.asarray(actions, dtype=np.float32)
    idx = np.asarray(policy_indices).astype(np.int64)
    W1 = np.ascontiguousarray(np.asarray(W1, dtype=np.float32))
    W2 = np.ascontiguousarray(np.asarray(W2, dtype=np.float32))
    W3 = np.ascontiguousarray(np.asarray(W3, dtype=np.float32))
    b1 = np.asarray(b1, dtype=np.float32)
    b2 = np.asarray(b2, dtype=np.float32)

    counts0 = np.bincount(idx, minlength=P)
    # group processing order: the group with the smallest past-512 remainder
    # runs LAST, so the endgame is [512, 512, tiny] with a minimal drain
    rem = [int(c) % 512 if int(c) % 512 else 512 for c in counts0]
    last = int(np.argmin(rem))
    order_g = [g for g in range(P) if g != last] + [last]
    rank = np.empty(P, dtype=np.int64)
    rank[order_g] = np.arange(P)
    order = np.argsort(rank[idx], kind="stable")
    counts = [int(counts0[g]) for g in order_g]

    in_maps = _pack_inputs(
        latents, actions, order, counts, order_g, W1, b1, W2, b2, W3
    )
    nc, tiles = _build_program(counts, latents.shape[0])
    return nc, in_maps, order, counts, tiles


def _scatter_out(results, order, tiles, B, policy_indices, b3):
    out = np.empty((B, D), dtype=np.float32)
    for i in range(NCORES):
        yS = np.asarray(results[i]["yS"], dtype=np.float32)  # [128, NT, 4, 64]
        ys = np.empty((B, Z), dtype=np.float32)
        for ti, (g, t0, tw) in enumerate(tiles):
            nch = -(-tw // 128)
            for c in range(nch):
                cw = min(128, tw - c * 128)
                ys[t0 + c * 128:t0 + c * 128 + cw] = yS[0:cw, ti, c, :]
        out[order, i * Z:(i + 1) * Z] = ys
    idx = np.asarray(policy_indices).astype(np.int64)
    out += np.asarray(b3, dtype=np.float32)[idx].reshape(B, D)
    return out


def kernel(latents, actions, policy_indices, W1, b1, W2, b2, W3, b3):
    global LAST_RESULT
    nc, in_maps, order, counts, tiles = _prepare(
        latents, actions, policy_indices, W1, b1, W2, b2, W3, b3
    )
    res = run_bass_kernel_spmd(nc, in_maps, list(range(NCORES)), trace=TRACE)
    LAST_RESULT = res
    return _scatter_out(
        res.results, order, tiles, np.asarray(latents).shape[0],
        policy_indices, b3,
    )


# revision 54
# speedup vs baseline: 1.0049x; 1.0041x over previous
"""Inner-policy-sharded Trainium2 kernel for DecoupledDynamicsModel (MoE).

Model: B=8192 rows; each row selects one of P=8 outer policies via
policy_indices; the selected policy runs 8 inner MLPs (72 -> 512 -> 512 -> 64)
on (latent chunk, action) and the 8 inner outputs concatenate to 512 dims.

Sharding: by INNER policy. Core i computes inner MLP i for every row, using
the row's outer-policy weight set W*[outer, i]. Rows are sorted by outer
policy on the host so tokens form 8 contiguous groups; within a group the
weights are stationary. Perfect load balance (every core runs exactly B
tokens), no capacity padding.

V2 layout (vs the f32r V1, 93230ns -> 86578ns cost-model):
  * All matmul operands are fp16 (1 cycle/row at any moving size, ~5e-4
    rel err): halves every DMA stream and lifts the f32r mult-of-4 /
    >=256-moving restrictions, so tokens need no padding at all.
  * Layer 3 (512 -> 64) runs TRANSPOSED: stationary = h2 token-chunk
    [128h x 128tok], moving = W3 k-chunk [128h x 64]. 16 matmuls of 64
    rows = 1024 cycles per 512-token tile instead of 4 x 512 = 2048 with
    the [64, N] orientation. Output lands token-major [tok, 64] in PSUM.
  * b1 is folded into the L1 matmul as a 73rd contraction row (x row 72
    is all-ones), so L1 evictions are bias-free; tiles <= 256 tokens pack
    multiple L1 m-chunks into one psum bank and evict with a single ACT
    op (kills the serial-eviction stall on small boundary/drain tiles).
  * b3 is not applied on device; the host adds it during the scatter.
  * Output DMAs are batched (4 token-tiles per dma_start): each HWDGE
    dma_start costs ~630ns descriptor-generation + ~900ns completion sem,
    so fewer/larger transfers matter more than latency. All weights and
    x are emitted in the preamble with zero sem waits, so the SP queue
    never head-of-line blocks behind eviction-gated output flushes.
  * Groups are processed in a permuted order chosen so the final group
    has the smallest past-512 remainder -> the drain tile is tiny.
  * A dummy activation at t=0 pulls the ~1.3us ACT function-table load
    into the DMA-startup dead window.

On-chip layout is feature-major ([features(part), tokens(free)]) so no
transposes are needed anywhere. Relu rides the PSUM->SBUF eviction:
ACT evicts layer 1, DVE evicts layers 2 and 3 (bias+relu via
tensor_scalar); both have slack vs the PE, which runs at ~87% occupancy
(75.5us busy, the fp16 4N+16N+2N rows/tile floor). On the last 3 tiles
the m0-2 L2 evictions move to ACT so the L3-gating m3 eviction starts on
DVE unqueued. L3 accumulates all token-chunks in ONE psum bank
(sequential start/stop groups, single eviction after the last matmul, so
no PE-write-vs-read bank hazard). PSUM: 4 L1 + 2 L2 + 2 L3 = 8 banks.
Built on Bacc so multi-wait instructions are legalized.
"""

import sys

sys.path.insert(0, "/opt/trn_rl_repo")

import numpy as np

import concourse.bass as bass
from concourse import bacc
import concourse.mybir as mybir
import concourse.tile as tile
from concourse.bass_utils import run_bass_kernel_spmd

P = 8          # outer policies == n_cores == inner MLPs per policy
Z = 64         # per-policy latent dim
D = P * Z      # 512
A = 8          # action dim
IN = Z + A     # 72, MLP input dim
H = 512        # hidden dim
NCORES = 8

F32 = mybir.dt.float32
F16 = mybir.dt.float16
RELU = mybir.ActivationFunctionType.Relu

TRACE = False
LAST_RESULT = None
YB = 4         # token tiles per batched output DMA

# schedule knobs (sweepable)
CFG = {
    "ps1": 4, "ps2": 2, "ps3": 2,   # PSUM bufs per layer (sum <= 8)
    "l3_pipe": False,                # emit L3(i) after L2(i+1)
    "y_queue": "sync",              # 'sync' | 'gpsimd'
    "ahead": 1,                      # (obsolete: all weights in preamble)
    "x_swdge_all": False,            # all x on SWDGE vs first span only
    "b12_swdge": True,               # biases via SWDGE (frees an HWDGE slot)
    "w2g0_3way": True,               # group-0 w2 as m0|m1|m23 for arrival
    "last_tile": 128,                # final tile size (drain length)
    "hbufs": 4,                      # h1/h2 pool buffers
    "ybufs": 4,                      # y-window pool buffers
    "yb": 4,                         # tiles per y window
    "l2_act": 0,                     # L2 evictions on ACT for m%2<l2_act
    "ytail_act": False,              # final tile y eviction on ACT (Copy)
    "tail_swdge": False,             # final flush DMA on SWDGE
    "esplit_tail": 0,                # m<esplit_m evicts on ACT, last N tiles
    "esplit_m": 3,
    "esplit_glast": False,           # esplit at every group's last tile
    "halve_rem": False,              # no tiny mid-run remainder tiles
    "x0_split": False,               # first SWDGE span = tile0 only
    "l3_pipe_n": 3,                  # L3-pipe only the last N tiles
    "b12_late": False,               # b12 after group-0 weights (L1 bias-free)
    "g0_tile": 512,                  # tile cap in the first group
    "end_tile": 512,                 # tile cap in the final group
}


def _group_tiles(counts):
    """Token tiles for the sorted stream: each tile stays inside one outer-
    policy group, <=512 tokens. The very last tile is kept small (<=128) so
    the end-of-kernel serial chain (L2 -> evict -> L3 -> evict -> DMA) is
    short."""
    tiles = []
    ng = len(counts)
    for g, n in enumerate(counts):
        off = sum(counts[:g])
        r = n
        first = True
        while r > 0:
            cap = 512
            if g == 0:
                cap = CFG["g0_tile"]
            elif g == ng - 1:
                cap = CFG["end_tile"]
            t = min(r, cap)
            # mid-run groups: halve 512..1024 remainders instead of leaving
            # a tiny trailing tile (tiny tiles stall on eviction latency)
            if CFG["halve_rem"] and g != ng - 1 and cap >= 512 and 512 < r <= 1024:
                t = -(-r // 2)
            # small leading tile: the first matmul is gated on a tiny x DMA
            if g == 0 and first and r > 384:
                t = 128
            # shorten the drain: final chunk of the final group is small
            lt = CFG["last_tile"]
            if g == ng - 1 and r <= cap and r > lt:
                t = min(t, r - lt)
            tiles.append((g, off, t))
            off += t
            r -= t
            first = False
    return tiles


def _build_program(counts, B):
    tiles = _group_tiles(counts)
    ntiles = len(tiles)
    YB = CFG["yb"]
    nc = bacc.Bacc()

    xTd = nc.declare_dram_parameter("xT", [IN + 1, B], F16, isOutput=False)
    w1d = nc.declare_dram_parameter("w1", [IN + 1, P, 512], F16, isOutput=False)
    w2d = nc.declare_dram_parameter("w2", [128, P, 4, 512], F16, isOutput=False)
    w3d = nc.declare_dram_parameter("w3", [128, P, 4, Z], F16, isOutput=False)
    b12d = nc.declare_dram_parameter("b12", [128, P, 8], F32, isOutput=False)
    # partition-major scrambled output: [partition, tile, tok-chunk, z]
    yd = nc.declare_dram_parameter("yS", [128, ntiles, 4, Z], F16, isOutput=True)

    # group boundaries in the sorted token stream
    gstart = [sum(counts[:g]) for g in range(P)]
    gend = [gstart[g] + counts[g] for g in range(P)]

    with tile.TileContext(nc) as tc:
        with (
            tc.tile_pool(name="w1p", bufs=8) as w1p,
            tc.tile_pool(name="w2p", bufs=8) as w2p,
            tc.tile_pool(name="w3p", bufs=8) as w3p,
            tc.tile_pool(name="bp", bufs=1) as bp,
            tc.tile_pool(name="xs", bufs=1) as xpool,
            tc.tile_pool(name="hs", bufs=CFG["hbufs"]) as hpool,
            tc.tile_pool(name="ys", bufs=CFG["ybufs"]) as ypool,
            tc.tile_pool(name="ps1", bufs=CFG["ps1"], space="PSUM") as pspool1,
            tc.tile_pool(name="ps2", bufs=CFG["ps2"], space="PSUM") as pspool2,
            tc.tile_pool(name="ps3", bufs=CFG["ps3"], space="PSUM") as pspool3,
        ):
            xt = xpool.tile([IN + 1, B], F16, tag="x")
            b12 = bp.tile([128, P, 8], F32, tag="b12")
            # dummy activation right at t=0: pulls the ~1.3us ACT
            # function-table load into the DMA-latency dead window
            warm = bp.tile([1, 1], F32, tag="warm")
            nc.vector.memset(warm, 0.0)
            nc.scalar.activation(warm[:, :], warm[:, :], RELU)
            w1s = [w1p.tile([IN + 1, 512], F16, tag="w1", name=f"w1_{g}") for g in range(P)]
            w2s = [w2p.tile([128, 4, 512], F16, tag="w2", name=f"w2_{g}") for g in range(P)]
            w3s = [w3p.tile([128, 4, Z], F16, tag="w3", name=f"w3_{g}") for g in range(P)]

            # --- DMA emission in need order ------------------------------
            # Every HWDGE dma_start costs ~630ns on the (serial) descriptor
            # generator plus ~900ns completion-sem latency, so batch
            # transfers wherever latency allows. ALL x columns ride SWDGE
            # (gpsimd), in spans sized so each tile's tokens land just
            # before its L1; the HWDGE pipe carries only weights + outputs.
            if CFG["x_swdge_all"]:
                xcuts = [0, tiles[0][2], 640, 2048, 4608, B]
                xcuts = sorted(set(min(c, B) for c in xcuts))
                for c0, c1 in zip(xcuts[:-1], xcuts[1:]):
                    if c1 > c0:
                        nc.gpsimd.dma_start(xt[:, c0:c1], xTd[:, c0:c1])
            else:
                sw_end = min(640, gend[0])
                if CFG["x0_split"] and tiles[0][2] < sw_end:
                    c = tiles[0][2]
                    nc.gpsimd.dma_start(xt[:, 0:c], xTd[:, 0:c])
                    nc.gpsimd.dma_start(xt[:, c:sw_end], xTd[:, c:sw_end])
                else:
                    nc.gpsimd.dma_start(xt[:, 0:sw_end], xTd[:, 0:sw_end])

            def emit_group_weights(g, split_w2=False):
                nc.sync.dma_start(w1s[g][:, :], w1d[:, g, :])
                if split_w2:
                    nc.sync.dma_start(w2s[g][:, 0, :], w2d[:, g, 0, :])
                    nc.sync.dma_start(w2s[g][:, 1:4, :], w2d[:, g, 1:4, :])
                else:
                    nc.sync.dma_start(w2s[g][:, :, :], w2d[:, g, :, :])
                nc.sync.dma_start(w3s[g][:, :, :], w3d[:, g, :, :])

            def emit_group_x(g):
                if not CFG["x_swdge_all"] and g < P:
                    lo = max(gstart[g], min(640, gend[0]) if g == 0 else gstart[g])
                    if g == 0:
                        lo = min(640, gend[0])
                    if lo < gend[g]:
                        nc.sync.dma_start(xt[:, lo:gend[g]], xTd[:, lo:gend[g]])

            # ENTIRE weight + x stream in the preamble: none of these DMAs
            # has a sem wait, so the SP queue never head-of-line blocks and
            # every group's weights are resident long before needed.
            nc.sync.dma_start(w1s[0][:, :], w1d[:, 0, :])
            if not CFG["b12_late"]:
                if CFG["b12_swdge"]:
                    nc.gpsimd.dma_start(b12[:, :, :], b12d[:, :, :])
                else:
                    nc.sync.dma_start(b12[:, :, :], b12d[:, :, :])
            nc.sync.dma_start(w2s[0][:, 0, :], w2d[:, 0, 0, :])
            if CFG["w2g0_3way"]:
                nc.sync.dma_start(w2s[0][:, 1, :], w2d[:, 0, 1, :])
                nc.sync.dma_start(w2s[0][:, 2:4, :], w2d[:, 0, 2:4, :])
            else:
                nc.sync.dma_start(w2s[0][:, 1:4, :], w2d[:, 0, 1:4, :])
            nc.sync.dma_start(w3s[0][:, :, :], w3d[:, 0, :, :])
            if CFG["b12_late"]:
                nc.sync.dma_start(b12[:, :, :], b12d[:, :, :])
            emit_group_x(0)
            for g in range(1, P):
                emit_group_x(g)
                emit_group_weights(g)

            # --- compute loop --------------------------------------------
            # Layer 3 of tile i is emitted after layer 2 of tile i+1: its
            # stationary operand is h2 (produced by the DVE eviction), so
            # the one-tile delay hides the eviction latency from the PE.
            # y-window boundaries: every YB tiles, plus a forced boundary so
            # the final (tiny) tile flushes alone -> short end-of-program DMA
            flush_after = {ti for ti in range(ntiles) if ti % YB == YB - 1}
            flush_after.add(ntiles - 1)
            if ntiles >= 2:
                flush_after.add(ntiles - 2)
            ystate = {"yt": None, "flushed": -1}

            def emit_l3(pg, ptw, ph2, pti):
                if pti % YB == 0:
                    ystate["yt"] = ypool.tile(
                        [128, YB, 4, Z], F16, tag="y", name=f"yt_{pti // YB}"
                    )
                yt = ystate["yt"]
                nch = -(-ptw // 128)
                # all tok-chunks accumulate into ONE psum bank (sequential
                # start/stop groups, no eviction until every chunk is done,
                # so no PE-write-vs-engine-read bank hazard), then a single
                # DVE eviction moves the whole [128, nch, 64] block out
                ps = pspool3.tile([128, 4, Z], F32, tag="ps3",
                                  name=f"ps3_{pti}")
                for c in range(nch):
                    cw = min(128, ptw - c * 128)
                    for k in range(4):
                        nc.tensor.matmul(
                            ps[0:cw, c, :],
                            ph2[:, k, c * 128:c * 128 + cw],
                            w3s[pg][:, k, :],
                            start=(k == 0),
                            stop=(k == 3),
                        )
                nc.vector.tensor_scalar(
                    yt[:, pti % YB, 0:nch, :], ps[:, 0:nch, :], 0.0, None,
                    mybir.AluOpType.add,
                )
                if pti in flush_after:
                    t_lo = ystate["flushed"] + 1
                    nb = pti - t_lo + 1
                    if nb > 0:
                        # mid-run flushes ride SWDGE (nothing queued behind
                        # their eviction waits); the final flush uses the
                        # now-empty SP queue for its lower latency
                        last_flush = pti == ntiles - 1
                        if last_flush:
                            dma = (nc.gpsimd.dma_start if CFG["tail_swdge"]
                                   else nc.sync.dma_start)
                        else:
                            dma = (nc.gpsimd.dma_start
                                   if CFG["y_queue"] == "gpsimd"
                                   else nc.sync.dma_start)
                        dma(
                            yd[:, t_lo:pti + 1, :, :],
                            yt[:, t_lo % YB:t_lo % YB + nb, :, :],
                        )
                        ystate["flushed"] = pti

            def emit_l1(g, t0, tw, ti):
                """Layer 1: h1 = relu(W1'.T @ x'), b1 folded in as the 73rd
                contraction row (x row 72 is all-ones). Returns the h1 tile."""
                h1 = hpool.tile([128, 4, 512], F16, tag="h1", name=f"h1_{ti}")
                if tw <= 256:
                    # small tile: m-chunks share psum banks, merged evicts
                    mper = 4 if tw <= 128 else 2
                    for m0 in range(0, 4, mper):
                        ps = pspool1.tile([128, mper, tw], F32, tag="ps1",
                                          name=f"ps1m_{ti}_{m0}")
                        for j in range(mper):
                            nc.tensor.matmul(
                                ps[:, j, :],
                                w1s[g][:, (m0 + j) * 128:(m0 + j + 1) * 128],
                                xt[:, t0:t0 + tw],
                                start=True,
                                stop=True,
                            )
                        nc.scalar.activation(
                            h1[:, m0:m0 + mper, :tw], ps[:, :, :], RELU
                        )
                else:
                    for m in range(4):
                        ps = pspool1.tile([128, 512], F32, tag="ps1",
                                          name=f"ps1_{ti}_{m}")
                        nc.tensor.matmul(
                            ps[:, :tw],
                            w1s[g][:, m * 128:(m + 1) * 128],
                            xt[:, t0:t0 + tw],
                            start=True,
                            stop=True,
                        )
                        nc.scalar.activation(h1[:, m, :tw], ps[:, :tw], RELU)
                return h1

            hoisted_h1 = {}
            pending = None
            prev_g = 0
            for (ti, (g, t0, tw)) in enumerate(tiles):


                h2 = hpool.tile([128, 4, 512], F16, tag="h2")
                if ti in hoisted_h1:
                    h1 = hoisted_h1.pop(ti)
                else:
                    h1 = emit_l1(g, t0, tw, ti)
                # hoist the drain tile's (tiny) L1 one tile early: it only
                # needs x + w1, and its ACT eviction then hides under this
                # tile's L2 compute instead of gating the drain tile's L2
                if ti + 1 == ntiles - 1 and ntiles >= 2:
                    ng, nt0, ntw = tiles[ti + 1]
                    hoisted_h1[ti + 1] = emit_l1(ng, nt0, ntw, ti + 1)
                # layer 2: h2 = relu(W2.T @ h1 + b2), K=512 over 4 chunks
                for m in range(4):
                    ps = pspool2.tile([128, 512], F32, tag="ps2")
                    for k in range(4):
                        nc.tensor.matmul(
                            ps[:, :tw],
                            w2s[g][:, m, k * 128:(k + 1) * 128],
                            h1[:, k, :tw],
                            start=(k == 0),
                            stop=(k == 3),
                        )
                    glast = ti + 1 == ntiles or tiles[ti + 1][0] != g
                    if (ti >= ntiles - CFG["esplit_tail"]
                            or (CFG["esplit_glast"] and glast))                             and m < CFG["esplit_m"]:
                        # endgame: m0-2 evictions ride the (idle) ACT so the
                        # L3-gating m3 eviction starts on DVE with no queue
                        nc.scalar.activation(
                            h2[:, m, :tw], ps[:, :tw], RELU,
                            bias=b12[:, g, 4 + m:5 + m],
                        )
                    elif m % 2 < CFG["l2_act"]:
                        nc.scalar.activation(
                            h2[:, m, :tw], ps[:, :tw], RELU,
                            bias=b12[:, g, 4 + m:5 + m],
                        )
                    else:
                        nc.vector.tensor_scalar(
                            h2[:, m, :tw],
                            ps[:, :tw],
                            b12[:, g, 4 + m:5 + m],
                            0.0,
                            mybir.AluOpType.add,
                            mybir.AluOpType.max,
                        )
                # layer 3 of the previous (or current) tile, see emit_l3
                if CFG["l3_pipe"] or ti >= ntiles - CFG["l3_pipe_n"]:
                    if pending is not None:
                        emit_l3(*pending)
                    pending = (g, tw, h2, ti)
                else:
                    emit_l3(g, tw, h2, ti)
            if pending is not None:
                emit_l3(*pending)

    nc.finalize()
    return nc, tiles


def _pack_inputs(latents, actions, order, counts, order_g, W1, b1, W2, b2, W3):
    """Per-core inputs. Core i: xT = [latent chunk i; action] for all rows in
    sorted order; weight tensors hold (outer g, inner i) stacks."""
    B = latents.shape[0]
    gsel = np.asarray(order_g)
    W1, W2, W3, b1, b2 = W1[gsel], W2[gsel], W3[gsel], b1[gsel], b2[gsel]
    lat_s = latents[order]                       # [B, 512]
    act_s = actions[order]                       # [B, 8]
    in_maps = []
    for i in range(NCORES):
        xT = np.empty((IN + 1, B), dtype=np.float16)
        xT[:Z] = lat_s[:, i * Z:(i + 1) * Z].T
        xT[Z:IN] = act_s.T
        xT[IN] = 1.0

        w1 = np.empty((IN + 1, P, 512), dtype=np.float16)
        w1[:IN] = W1[:, i].transpose(1, 0, 2).astype(np.float16)
        w1[IN] = b1[:, i].astype(np.float16)     # b1 folded as K-row 72
        # w2[p, g, m, k*128+q] = W2[g, i, k*128+p, m*128+q]
        w2 = np.ascontiguousarray(
            W2[:, i].reshape(P, 4, 128, 4, 128)   # g, k, p, m, q
            .transpose(2, 0, 3, 1, 4)             # p, g, m, k, q
            .reshape(128, P, 4, 512)
        ).astype(np.float16)
        # w3[p, g, k, z] = W3[g, i, k*128+p, z]
        w3 = np.ascontiguousarray(
            W3[:, i].reshape(P, 4, 128, Z).transpose(2, 0, 1, 3)
        ).astype(np.float16)
        b12 = np.empty((128, P, 8), dtype=np.float32)
        b12[:, :, 0:4] = b1[:, i].reshape(P, 4, 128).transpose(2, 0, 1)
        b12[:, :, 4:8] = b2[:, i].reshape(P, 4, 128).transpose(2, 0, 1)

        in_maps.append({"xT": xT, "w1": w1, "w2": w2, "w3": w3, "b12": b12})
    return in_maps


def _prepare(latents, actions, policy_indices, W1, b1, W2, b2, W3, b3):
    latents = np.asarray(latents, dtype=np.float32)
    actions = np.asarray(actions, dtype=np.float32)
    idx = np.asarray(policy_indices).astype(np.int64)
    W1 = np.ascontiguousarray(np.asarray(W1, dtype=np.float32))
    W2 = np.ascontiguousarray(np.asarray(W2, dtype=np.float32))
    W3 = np.ascontiguousarray(np.asarray(W3, dtype=np.float32))
    b1 = np.asarray(b1, dtype=np.float32)
    b2 = np.asarray(b2, dtype=np.float32)

    counts0 = np.bincount(idx, minlength=P)
    # group processing order: the group with the smallest past-512 remainder
    # runs LAST, so the endgame is [512, 512, tiny] with a minimal drain
    rem = [int(c) % 512 if int(c) % 512 else 512 for c in counts0]
    last = int(np.argmin(rem))
    order_g = [g for g in range(P) if g != last] + [last]
    rank = np.empty(P, dtype=np.int64)
    rank[order_g] = np.arange(P)
    order = np.argsort(rank[idx], kind="stable")
    counts = [int(counts0[g]) for g in order_g]

    in_maps = _pack_inputs(
        latents, actions, order, counts, order_g, W1, b1, W2, b2, W3
    )
    nc, tiles = _build_program(counts, latents.shape[0])
    return nc, in_maps, order, counts, tiles


def _scatter_out(results, order, tiles, B, policy_indices, b3):
    out = np.empty((B, D), dtype=np.float32)
    for i in range(NCORES):
        yS = np.asarray(results[i]["yS"], dtype=np.float32)  # [128, NT, 4, 64]
        ys = np.empty((B, Z), dtype=np.float32)
        for ti, (g, t0, tw) in enumerate(tiles):
            nch = -(-tw // 128)
            for c in range(nch):
                cw = min(128, tw - c * 128)
                ys[t0 + c * 128:t0 + c * 128 + cw] = yS[0:cw, ti, c, :]
        out[order, i * Z:(i + 1) * Z] = ys
    idx = np.asarray(policy_indices).astype(np.int64)
    out += np.asarray(b3, dtype=np.float32)[idx].reshape(B, D)
    return out


def kernel(latents, actions, policy_indices, W1, b1, W2, b2, W3, b3):
    global LAST_RESULT
    nc, in_maps, order, counts, tiles = _prepare(
        latents, actions, policy_indices, W1, b1, W2, b2, W3, b3
    )
    res = run_bass_kernel_spmd(nc, in_maps, list(range(NCORES)), trace=TRACE)
    LAST_RESULT = res
    return _scatter_out(
        res.results, order, tiles, np.asarray(latents).shape[0],
        policy_indices, b3,
    )
